# revision 1
# baseline (speedup 1.0000x reference)
"""Trainium2 Bass kernel for BilinearInteractionLayer (B=8192, F=32, E=64).

out[b, p, :] = (x[b, i_p, :] @ W) * x[b, j_p, :] for the 496 upper-triangle
field pairs (i < j), computed data-parallel over the batch on 8 NeuronCores
(1024 batches per core), W replicated.

Per-core kernel (batch on SBUF partitions throughout):
  - stream x in 128-batch tiles [128, 2048] (contiguous 1 MiB DMAs, prefetch
    depth 2)
  - project on PE, 2 fields per pass: transpose [128,128] block -> PSUM,
    copy to SBUF (ACT), matmul against block-diag(W, W) -> xw in PSUM
  - evacuate xw PSUM -> SBUF (ACT) so the next tile's matmuls reuse PSUM
  - one DVE tensor_mul per left field i (stride-0 broadcast of xw_i across
    the j range) writes output chunks in SBUF
  - chunked DMA stores, contiguous (pairs, E) runs per batch row

The layer is memory-bound: ~132 MiB of HBM traffic per core, dominated by
the 124 MiB output shard. Everything else overlaps the store stream.
"""

import sys

if "/opt/trn_rl_repo" not in sys.path:
    sys.path.insert(0, "/opt/trn_rl_repo")

import numpy as np

B, F, E = 8192, 32, 64
N_CORES = 8
B_LOCAL = B // N_CORES
NPAIR = F * (F - 1) // 2
P = 128

_nc_cache = {}


def _pair_chunks():
    """Group left-field blocks (i has F-1-i pairs) into store chunks of
    ~64 pairs (8 stores of ~2 MB per tile): fine enough that the DVE stays
    ahead of the store stream, big enough for full DMA efficiency."""
    chunks, cur, cur_n, off = [], [], 0, 0
    for i in range(F - 1):
        cur.append(i)
        cur_n += F - 1 - i
        if cur_n >= 50 or i == F - 2:
            chunks.append((tuple(cur), off, cur_n))
            off += cur_n
            cur, cur_n = [], 0
    return chunks


def _build_nc(hw_loop=0):
    """hw_loop > 0 wraps the whole kernel body in a For_i hardware loop that
    re-runs it hw_loop times — used only by test.py to measure HW exec time
    as a wall-clock delta between two loop counts."""
    import concourse.bacc as bacc
    import concourse.bass as bass
    import concourse.mybir as mybir
    from concourse.masks import make_identity
    from concourse.tile import TileContext

    F32 = mybir.dt.float32
    nb = B_LOCAL // P
    prefetch = 4

    nc = bacc.Bacc("TRN2", target_bir_lowering=False, debug=False,
                   num_devices=N_CORES)
    x = nc.declare_dram_parameter("x", [B_LOCAL, F, E], F32, isOutput=False)
    w = nc.declare_dram_parameter("W", [E, E], F32, isOutput=False)
    out = nc.declare_dram_parameter("out", [B_LOCAL, NPAIR, E], F32,
                                    isOutput=True)
    chunks = _pair_chunks()

    with TileContext(nc) as tc:
        with (
            tc.tile_pool(name="consts", bufs=1) as consts,
            tc.tile_pool(name="xload", bufs=prefetch + 1) as xpool,
            tc.tile_pool(name="xtsb", bufs=3) as xtp,
            tc.tile_pool(name="xwsb", bufs=2) as xwp,
            tc.tile_pool(name="outc", bufs=7) as outp,
            tc.tile_pool(name="ptr", bufs=3, space="PSUM") as ptr,
            tc.tile_pool(name="pxw", bufs=1, space="PSUM") as pxw,
        ):
            ident = consts.tile([P, P], F32)
            make_identity(nc, ident[:])
            w2 = consts.tile([P, P], F32)
            nc.gpsimd.memset(w2[:], 0.0)
            nc.sync.dma_start(out=w2[0:E, 0:E], in_=w.ap())
            nc.sync.dma_start(out=w2[E:2 * E, E:2 * E], in_=w.ap())

            x_flat = x.ap().rearrange("b f e -> b (f e)")
            out_ap = out.ap()
            loaded = {}

            def load(t):
                x_sb = xpool.tile([P, F * E], F32, tag="x_sb")
                # loads go through the Pool/SWDGE path so they never queue
                # ahead of store chunks in the SP HWDGE rotation
                nc.gpsimd.dma_start(out=x_sb[:],
                                    in_=x_flat[t * P:(t + 1) * P, :])
                loaded[t] = x_sb

            def btile(t):
                x_sb = loaded.pop(t)

                xw_ps = pxw.tile([P, F * E], F32, tag="xw_ps")
                xw_sb = xwp.tile([P, F * E], F32, tag="xw_sb")
                q = F * E // 4
                for fg in range(F // 2):
                    xT_ps = ptr.tile([P, P], F32, tag="xT_ps")
                    nc.tensor.transpose(
                        xT_ps[:], x_sb[:, fg * P:(fg + 1) * P], ident[:])
                    xT_sb = xtp.tile([P, P], F32, tag="xT_sb")
                    nc.scalar.copy(xT_sb[:], xT_ps[:])
                    nc.tensor.matmul(
                        xw_ps[:, fg * P:(fg + 1) * P],
                        lhsT=xT_sb[:], rhs=w2[:], start=True, stop=True)
                    if fg % 4 == 3:
                        # evacuate each xw quarter as soon as its matmuls
                        # land so the first chunk's muls start early and
                        # the PSUM banks free up for the next tile
                        s = fg // 4
                        nc.scalar.copy(xw_sb[:, s * q:(s + 1) * q],
                                       xw_ps[:, s * q:(s + 1) * q])

                for (i_list, p_off, npc) in chunks:
                    och = outp.tile([P, npc * E], F32, tag="och")
                    loc = 0
                    for i in i_list:
                        nj = F - 1 - i
                        in0 = xw_sb[:, i * E:(i + 1) * E].rearrange(
                            "p (j e) -> p j e", j=1)
                        in1 = x_sb[:, (i + 1) * E:F * E].rearrange(
                            "p (j e) -> p j e", e=E)
                        o = och[:, loc * E:(loc + nj) * E].rearrange(
                            "p (j e) -> p j e", e=E)
                        in0b, _ = bass.broadcast_tensor_aps(in0, in1)
                        nc.vector.tensor_mul(o, in0b, in1)
                        loc += nj
                    nc.sync.dma_start(
                        out=out_ap[t * P:(t + 1) * P, p_off:p_off + npc, :],
                        in_=och[:])

            def run_all():
                for t in range(min(prefetch, nb)):
                    load(t)
                for t in range(nb):
                    if t + prefetch < nb:
                        load(t + prefetch)
                    btile(t)

            if hw_loop:
                with tc.For_i(0, hw_loop, 1):
                    run_all()
            else:
                run_all()

    nc.compile()
    return nc


def kernel(x, W):
    from concourse.bass_utils import run_bass_kernel_spmd

    x = np.ascontiguousarray(np.asarray(x, dtype=np.float32))
    W = np.ascontiguousarray(np.asarray(W, dtype=np.float32))
    assert x.shape == (B, F, E) and W.shape == (E, E)

    if "nc" not in _nc_cache:
        _nc_cache["nc"] = _build_nc()
    nc = _nc_cache["nc"]

    in_maps = [
        {"x": x[c * B_LOCAL:(c + 1) * B_LOCAL], "W": W}
        for c in range(N_CORES)
    ]
    res = run_bass_kernel_spmd(nc, in_maps, list(range(N_CORES)))
    return np.concatenate([res.results[c]["out"] for c in range(N_CORES)],
                          axis=0)


if __name__ == "__main__":
    rng = np.random.default_rng(0)
    x = rng.standard_normal((B, F, E)).astype(np.float32)
    W = (rng.standard_normal((E, E)) / np.sqrt(E)).astype(np.float32)
    got = kernel(x=x, W=W)
    i_idx, j_idx = np.triu_indices(F, k=1)
    exp = np.einsum("bfe,ed->bfd", x, W)[:, i_idx, :] * x[:, j_idx, :]
    err = np.abs(got - exp).max()
    print("max abs err:", err, "rel:", err / np.abs(exp).max())

